# revision 49
# baseline (speedup 1.0000x reference)
"""Causal self-attention (B=2, S=2048, D=1024, H=16, Hd=64) on 8 TRN2 NeuronCores.

Sharding: tensor-parallel over heads (4 heads/core) x data-parallel over batch
(cores 0-3 -> batch 0, cores 4-7 -> batch 1). Each core:
  - computes q^T,k^T (transposed layout, heads stacked in pairs on partitions)
    and v (natural layout) for its 4 heads
  - runs causal attention in transposed-score layout (scores_T[k, q]) so no
    transposes are ever needed; the v tiles carry 64 identical ones-columns,
    so each attnV matmul deposits 64 broadcast copies of the softmax
    denominator on PSUM partitions 0-63 for free -- normalization is then a
    single fast-approx DVE reciprocal over [64,512] + one DVE multiply
  - computes its partial output projection y_part = out_heads @ W_proj[rows]
Host sums the 4 bf16 partials per batch and adds b_proj (the unshard step for
a row-parallel matmul). Matmul datapath is bf16 (fp32 PSUM accumulation).

Schedule: software-pipelined around two hardware behaviors measured in the
NTFF traces: (1) the PE's HAM clock gate halves the clock after any ~idle
window, so the schedule keeps PE activity continuous -- dummy warm-up/
warm-keeper matmuls bridge the DMA load window and each chunk-boundary
normalize lull; (2) cross-engine round-trips (attnV PSUM WAR on the previous
chunk's normalize) are covered by "filler" units -- the qkv / v / output-
projection matmuls split into quarter-granular generators interleaved
between attention steps, paced by estimated PE-ns. attnV trails its exp by
three steps so it never waits on ScalarE. All eight hp=0 qk quarters are
computed during the input-DMA window (two accumulation waves, one stream per
PSUM bank, wave 1 trailing by one k-tile), so dense attention starts the
moment the load finishes, already at the warm 2.4 GHz clock. The long
(jj=3) chunks run first while filler is plentiful; the short jj=0 chunks
form the tail, with their projection evictions moved to the then-idle
ScalarE and y stored as full 2 KB rows.
"""

import sys

if "/opt/trn_rl_repo" not in sys.path:
    sys.path.insert(0, "/opt/trn_rl_repo")

import ml_dtypes
import numpy as np


def _ensure_axon_hooks():
    """bass_utils imports antenv.axon_hooks when tracing is requested; the
    slim agent image lacks it. Provide the real ctypes hook if possible,
    else a None-returning stub (bass_utils then skips tracing gracefully)."""
    try:
        import antenv.axon_hooks  # noqa: F401

        return
    except ImportError:
        pass
    import types

    hook = None
    try:
        from trn_agent_boot.trn_boot import _ntff_profile_via_ctypes

        hook = _ntff_profile_via_ctypes("/opt/axon/libaxon_pjrt.so")
    except Exception:
        pass
    mod = types.ModuleType("antenv.axon_hooks")
    mod.get_axon_ntff_profile_hook = lambda: hook
    mod.set_axon_ntff_profile_hook = lambda h: None
    sys.modules["antenv.axon_hooks"] = mod


_ensure_axon_hooks()

D = 1024
S = 2048
B = 2
H = 16
HD = 64
N_CORES = 8
GROUPS = 4  # cores per batch
HPC = 4  # heads per core
SCALE = 1.0 / np.sqrt(HD)
KT = D // 128  # 8 contraction tiles
ST = S // 128  # 16 seq tiles

_module_cache = {}


def _build_module():
    if "nc" in _module_cache:
        return _module_cache["nc"]

    import concourse.bacc as bacc
    import concourse.mybir as mybir
    import concourse.tile as tile
    from concourse.bass import ts

    f32 = mybir.dt.float32
    bf16 = mybir.dt.bfloat16
    AF = mybir.ActivationFunctionType

    nc = bacc.Bacc("TRN2", target_bir_lowering=False, debug=False)

    xT = nc.dram_tensor("xT", [D, S], bf16, kind="ExternalInput")
    w_qk = nc.dram_tensor("w_qk", [D, 512], bf16, kind="ExternalInput")
    b_qk = nc.dram_tensor("b_qk", [128, 4], f32, kind="ExternalInput")
    w_v = nc.dram_tensor("w_v", [D, 256], bf16, kind="ExternalInput")
    b_v = nc.dram_tensor("b_v", [128, 256], f32, kind="ExternalInput")
    w_pr = nc.dram_tensor("w_pr", [256, D], bf16, kind="ExternalInput")
    y = nc.dram_tensor("y", [S, D], bf16, kind="ExternalOutput")

    import contextlib

    with tile.TileContext(nc) as tc:
        with contextlib.ExitStack() as ctx2:
            const = ctx2.enter_context(tc.tile_pool(name="const", bufs=1))
            # ---- resident SBUF tensors ----
            xT_sb = const.tile([128, KT, S], bf16)
            wqk_sb = const.tile([128, KT, 512], bf16)
            wv_sb = const.tile([128, KT, 256], bf16)
            bqk_sb = const.tile([128, 4], f32)
            bv_sb = const.tile([128, 4, 64], f32)
            wpr_sb = const.tile([128, 2, D], bf16)
            ones_sb = const.tile([1, 64], f32)
            warm_sb = const.tile([1, 64], f32)
            qkT_sb = const.tile([128, 4, S], bf16)  # m: q01,q23,k01,k23
            wu_sb = const.tile([128, 512], bf16)  # HAM warmup operand
            v_sb = const.tile([128, ST, 4, 128], bf16)  # per head: [ones|63 pad|V]
            oT_sb = const.tile([128, 2, S], bf16)  # normalized attn out

            # The input load is issue-limited on a single queue (measured
            # ~290GB/s avg vs ~400GB/s fabric peak): split wqk/xT across the
            # sync + scalar queues (alternate k-tiles), wv behind on both,
            # wpr last. ACT queue is idle during the load so borrowing it is
            # free; exp work only starts ~28us in.
            nc.vector.memset(ones_sb[:], 1.0)
            nc.vector.memset(wu_sb[:], 0.03)
            # preload the ACT exp table set early, off the critical path
            nc.scalar.activation(warm_sb[:], ones_sb[:], AF.Exp)
            nc.sync.dma_start(out=bqk_sb[:], in_=b_qk[:])
            nc.sync.dma_start(out=bv_sb[:], in_=b_v[:])
            qs = (nc.sync, nc.scalar)
            for k in range(KT):
                qs[k % 2].dma_start(out=wqk_sb[:, k, :], in_=w_qk[ts(k, 128), :])
                qs[(k + 1) % 2].dma_start(out=xT_sb[:, k, :], in_=xT[ts(k, 128), :])
            for k in range(KT):
                qs[k % 2].dma_start(out=wv_sb[:, k, :], in_=w_v[ts(k, 128), :])
            nc.sync.dma_start(out=wpr_sb[:, 0, :], in_=w_pr[0:128, :])
            nc.scalar.dma_start(out=wpr_sb[:, 1, :], in_=w_pr[128:256, :])
            for h in range(HPC):
                # 64 identical ones-columns: the attnV matmul then deposits
                # 64 copies of the softmax denominator on PSUM partitions
                # 0-63 (same streamed column count, so zero extra PE cost),
                # and V lands on partitions 64-127. Normalization is then
                # just reciprocal_approx_fast over [64,512] + one DVE mul --
                # no partition_broadcast (gpsimd scheduling is erratic) and
                # no PSUM+PSUM operand pairs.
                nc.gpsimd.memset(v_sb[:, :, h, 0:64], 1.0)

            # PSUM budget (8 banks): scores 2 bufs x [128,2,512] = 4 banks,
            # attnV accumulators 2 x [65,512] = 2 banks, filler [128,1024]
            # = 2 banks.
            psS = ctx2.enter_context(tc.tile_pool(name="psS", bufs=2, space="PSUM"))
            psO = ctx2.enter_context(tc.tile_pool(name="psO", bufs=1, space="PSUM"))
            psF = ctx2.enter_context(tc.tile_pool(name="psF", bufs=2, space="PSUM"))
            ptp = ctx2.enter_context(tc.tile_pool(name="pt", bufs=6))
            ysbp = ctx2.enter_context(tc.tile_pool(name="ysb", bufs=3))
            nrm = ctx2.enter_context(tc.tile_pool(name="nrm", bufs=2))

            # ---- filler units: generators yielding approx PE-ns per matmul
            def gen_qk_quarter(m, q2):
                """qkT_sb[:, m, q2*512:...] = (x @ w_qk[:, m-tile]) + bias."""
                ps = psF.tile([128, 512], f32, tag="f", name="ps_qk")
                for k in range(KT):
                    nc.tensor.matmul(
                        ps[:, 0:512],
                        lhsT=wqk_sb[:, k, ts(m, 128)],
                        rhs=xT_sb[:, k, ts(q2, 512)],
                        start=(k == 0),
                        stop=(k == KT - 1),
                    )
                    yield 216.0
                nc.vector.tensor_scalar_add(
                    qkT_sb[:, m, ts(q2, 512)], ps[:, 0:512], bqk_sb[:, m : m + 1]
                )
                yield 0.0

            def gen_v_quarter(qt):
                """v_sb seq-tiles 2qt, 2qt+1 (natural layout, + bias)."""
                ps = psF.tile([128, 2, 4, 64], f32, tag="f", name="ps_v")
                for sti in range(2):
                    st = qt * 2 + sti
                    for k in range(KT):
                        nc.tensor.matmul(
                            ps[:, sti],
                            lhsT=xT_sb[:, k, ts(st, 128)],
                            rhs=wv_sb[:, k, :],
                            start=(k == 0),
                            stop=(k == KT - 1),
                        )
                        yield 110.0
                    nc.vector.tensor_add(
                        v_sb[:, st, :, 64:128], ps[:, sti], bv_sb[:]
                    )
                    yield 0.0

            def gen_proj(m, evict_on_act=False):
                """y[m-tile, :] = oT[m-tile]^T @ w_pr. Both 512-col halves
                land in one [128,1024] staging tile so the store is a single
                DMA with full 2KB DRAM lines (512-col stores ran at ~half
                fabric rate on 1KB lines)."""
                y_sb = ysbp.tile([128, 1024], bf16, tag="ysb", name="y_sb")
                for nch in range(2):
                    ps = psF.tile([128, 512], f32, tag="f", name="ps_y")
                    for kp in range(2):
                        nc.tensor.matmul(
                            ps[:, 0:512],
                            lhsT=oT_sb[:, kp, ts(m, 128)],
                            rhs=wpr_sb[:, kp, ts(nch, 512)],
                            start=(kp == 0),
                            stop=(kp == 1),
                        )
                        yield 216.0
                    if evict_on_act:
                        # late proj units run while ACT is idle and DVE is
                        # the endgame bottleneck (normalize muls + evictions)
                        nc.scalar.activation(y_sb[:, ts(nch, 512)], ps[:], AF.Copy)
                    else:
                        nc.vector.tensor_scalar_add(
                            y_sb[:, ts(nch, 512)], ps[:], 0.0
                        )
                nc.sync.dma_start(out=y[ts(m, 128), :], in_=y_sb[:])
                yield 0.0

            class Filler:
                """Queue of named filler units (generators of PE matmuls).
                pull(ns) paces emission by approximate PE time; ensure(key)
                force-emits a unit NOW (producers must be emitted before
                their consumers — units are mutually independent)."""

                def __init__(self):
                    self.queue = []  # list of (key, gen)
                    self.cur = None  # (key, gen) partially emitted

                def add(self, key, gen):
                    self.queue.append((key, gen))

                def ensure(self, key):
                    if self.cur is not None and self.cur[0] == key:
                        for _ in self.cur[1]:
                            pass
                        self.cur = None
                        return
                    for i, (k, g) in enumerate(self.queue):
                        if k == key:
                            del self.queue[i]
                            for _ in g:
                                pass
                            return

                def pull(self, ns):
                    while ns > 0:
                        if self.cur is None:
                            if not self.queue:
                                return
                            self.cur = self.queue.pop(0)
                        try:
                            ns -= next(self.cur[1])
                        except StopIteration:
                            self.cur = None

                def drain(self):
                    while self.cur is not None or self.queue:
                        self.pull(1e12)

            filler = Filler()

            # ---- attention chunk: 512 q-cols of one head pair ----
            def attn_chunk(
                hp, jj, pull_scale, start_pull=0.0, norm_split=False,
                add_after_start=(),
            ):
                c0 = 512 * jj
                n_sk = 4 * jj + 4
                po = [
                    psO.tile([128, 512], f32, tag=f"o{h}", name=f"po{h}")
                    for h in range(2)
                ]

                def emit_attnv(sk, pt, col0, n):
                    for h in range(2):
                        nc.tensor.matmul(
                            po[h][:, col0 - c0 : col0 - c0 + n],
                            lhsT=v_sb[:, sk, 2 * hp + h, :],
                            rhs=pt[:, h, 0:n],
                            start=(sk == 0),
                            stop=(sk == n_sk - 1),
                        )

                filler.ensure(("qk", hp, jj))  # this chunk's q quarter
                filler.ensure(("qk", 2 + hp, 0))  # first k quarter
                # Boundary burst: the first attnV of this chunk reuses the
                # previous chunk's po PSUM banks (psO bufs=1), so it WAR-waits
                # on the prev normalize chain (reciprocal -> mul on DVE).
                # Emit enough filler ahead of it that the PE never drains
                # while that chain runs. The previous pair's proj units are
                # only added AFTER this burst: their oT LDWEIGHTS waits on
                # the muls just emitted, so pulling one inside the burst
                # would stall the PE queue on the very chain we're covering.
                filler.pull(start_pull)
                for key, gen in add_after_start:
                    filler.add(key, gen)
                pend = []
                for sk in range(n_sk):
                    col0 = max(c0, sk * 128)
                    n = c0 + 512 - col0
                    filler.ensure(("qk", 2 + hp, sk // 4))  # k quarter
                    ps = psS.tile([128, 2, 512], f32, tag="s", name="ps")
                    for h in range(2):
                        hr = h * 64
                        nc.tensor.matmul(
                            ps[:, h, 0:n],
                            lhsT=qkT_sb[hr : hr + 64, 2 + hp, ts(sk, 128)],
                            rhs=qkT_sb[hr : hr + 64, hp, col0 : col0 + n],
                            start=True,
                            stop=True,
                            tile_position=(hr, 0),
                        )
                    pt = ptp.tile([128, 2, 512], bf16, tag="pt", name="pt")
                    if n == 512:
                        nc.scalar.activation(pt[:], ps[:], AF.Exp)
                    else:
                        nc.scalar.activation(pt[:, :, 0:n], ps[:, :, 0:n], AF.Exp)
                    if col0 == sk * 128:
                        # causal mask: zero pt below the diagonal on the
                        # (idle) gpsimd engine; the one-step delay before
                        # attnV consumes pt hides the latency, and it keeps
                        # the 64 mask matmuls off the bottleneck PE.
                        for h in range(2):
                            nc.gpsimd.affine_select(
                                out=pt[:, h, 0:128],
                                in_=pt[:, h, 0:128],
                                compare_op=mybir.AluOpType.is_ge,
                                fill=0.0,
                                base=0,
                                pattern=[[1, 128]],
                                channel_multiplier=-1,
                            )
                    # filler between this step's scores and the previous
                    # step's attnV: covers the exp latency on the PE queue.
                    act_ns = (2 * n + 352) / 1.2
                    attn_pe = (3 * n) / 2.4 + 60
                    filler.pull(max(150.0, (act_ns - attn_pe) * pull_scale))
                    # two-step attnV delay: attnV for step N is emitted
                    # during step N+2, so it never waits on the exp stream,
                    # and a chunk's first attnV sits two steps past the
                    # boundary (more slack for the po WAR on the previous
                    # chunk's normalize).
                    if len(pend) == 3:
                        p = pend.pop(0)
                        filler.ensure(("v", p[0] // 2))
                        emit_attnv(*p)
                    pend.append((sk, pt, col0, n))
                for p in pend:
                    filler.ensure(("v", p[0] // 2))
                    emit_attnv(*p)
                # A filler slab BEFORE the normalize ops: these fillers'
                # DVE evictions enter the queue ahead of rec/mul (which wait
                # on the attnV just emitted), so the psF-bank WAR for the
                # next fillers never stalls the PE behind a blocked DVE head.
                filler.pull(1000.0)
                # normalization: partitions 0-63 of po hold 64 identical
                # copies of the denominator (ones-columns in the v tile), so
                # 1/denom lands broadcast-ready in one reciprocal, and one
                # DVE mul (PSUM x SBUF) finishes the chunk. Deterministic
                # ~1.4us/head, all on the in-order DVE queue. For the final
                # pair (norm_split) the muls go out in 128-col slices,
                # head-interleaved, so each proj m-tile can start as soon as
                # its slice of oT is normalized.
                rbcs = []
                if norm_split:
                    for h in range(2):
                        rbc = nrm.tile([64, 512], f32, tag=f"rbc{h}", name="rbc")
                        nc.vector.reciprocal_approx_fast(
                            out=rbc[:], in_=po[h][0:64, :]
                        )
                        rbcs.append(rbc)
                    for wi in range(8):
                        nc.tensor.matmul(
                            po[wi % 2][0:64, :],
                            lhsT=wu_sb[:, 0:64],
                            rhs=wu_sb[:],
                            start=True,
                            stop=True,
                        )
                    for p4 in range(4):
                        sl = slice(128 * p4, 128 * (p4 + 1))
                        for h in range(2):
                            hr = h * 64
                            nc.vector.tensor_mul(
                                oT_sb[hr : hr + 64, hp, c0 + 128 * p4 : c0 + 128 * (p4 + 1)],
                                po[h][64:128, sl],
                                rbcs[h][:, sl],
                            )
                else:
                    for h in range(2):
                        rbc = nrm.tile([64, 512], f32, tag=f"rbc{h}", name="rbc")
                        nc.vector.reciprocal_approx_fast(
                            out=rbc[:], in_=po[h][0:64, :]
                        )
                        rbcs.append(rbc)
                    # HAM warm-keepers: dummy MMs into the dead denominator
                    # region (rec already consumed it). They WAR-wait the
                    # recs, so they execute exactly inside the boundary lull.
                    # Full 512-col streams: 64-col keepers pipeline at ~32ns
                    # apart and give only ~0.3us of activity, too little for
                    # the ~1.7us lull the HAM MID window sees.
                    for wi in range(6):
                        nc.tensor.matmul(
                            po[wi % 2][0:64, :],
                            lhsT=wu_sb[:, 0:64],
                            rhs=wu_sb[:],
                            start=True,
                            stop=True,
                        )
                    for h in range(2):
                        hr = h * 64
                        nc.vector.tensor_mul(
                            oT_sb[hr : hr + 64, hp, c0 : c0 + 512],
                            po[h][64:128, :],
                            rbcs[h][:],
                        )

            # ---- HAM warmup: dense dummy matmuls at t=0 (PE is otherwise
            # idle waiting on DMA, and would run cold at 1.2 GHz otherwise).
            ps_w = psF.tile([128, 512], f32, tag="f", name="ps_warm")
            for i in range(10):
                nc.tensor.matmul(
                    ps_w[:, 0:512],
                    lhsT=wu_sb[:, 0:128],
                    rhs=wu_sb[:],
                    start=(i == 0),
                    stop=(i == 9),
                )

            # ---- ramp: ALL 8 hp=0 qk quarters (q01 + k01 over the full
            # sequence), k-pipelined with the input DMA stream. Both waves
            # interleave per k-tile (8 MMs / ~1.7us of PE per ~1.6us k-tile
            # arrival) so the PE runs at ~full duty through the load window
            # and HAM never re-throttles. Wave 0 (q2 0,1) accumulates in the
            # psS banks; wave 1 (q2 2,3) in 256-col half-groups placed in
            # the otherwise-idle psO/psF banks (1 bank each). The psF buf0
            # group is emitted last within each k so its WAR on the warmup
            # accumulator clears before it reaches the queue head.
            # One accumulation stream per PSUM bank: interleaved start=True
            # groups sharing a bank corrupt each other (the has_written
            # clear is bank-wide), so each (m, q2) stream gets a whole bank.
            psA = psS.tile([128, 2, 512], f32, tag="s", name="rampA")
            psB = psS.tile([128, 2, 512], f32, tag="s", name="rampB")
            w1_streams = [
                (0, 2, psO.tile([128, 512], f32, tag="o0", name="rampO0")),
                (2, 2, psO.tile([128, 512], f32, tag="o1", name="rampO1")),
                (0, 3, psF.tile([128, 512], f32, tag="f", name="rampF1")),
                (2, 3, psF.tile([128, 512], f32, tag="f", name="rampF0")),
            ]
            # wave1 trails wave0 by 3 k-tiles: during the cold-clock window
            # only wave0 runs (4 cold MMs/k-tile ~= the DMA arrival rate);
            # once HAM is warm, 8 MMs/k-tile still matches arrivals, and the
            # last 3 wave1 k-groups run dense right after the load.
            def w0_mms(k):
                for q2i, q2 in enumerate((0, 1)):
                    pst = (psA, psB)[q2i]
                    for mi, m in enumerate((0, 2)):
                        nc.tensor.matmul(
                            pst[:, mi, :],
                            lhsT=wqk_sb[:, k, ts(m, 128)],
                            rhs=xT_sb[:, k, ts(q2, 512)],
                            start=(k == 0),
                            stop=(k == KT - 1),
                        )

            def w1_mms(k):
                for m, q2, pst in w1_streams:
                    nc.tensor.matmul(
                        pst[:],
                        lhsT=wqk_sb[:, k, ts(m, 128)],
                        rhs=xT_sb[:, k, ts(q2, 512)],
                        start=(k == 0),
                        stop=(k == KT - 1),
                    )

            for k in range(KT):
                w0_mms(k)
                if k >= 1:
                    w1_mms(k - 1)
            w1_mms(KT - 1)
            # evictions in first-needed order: chunk (0,3) sk=0 reads the
            # k01/q2=0 quarter (psA mi=1) as lhsT and the q01/q2=3 quarter
            # (wave1 stream 2) as rhs.
            nc.vector.tensor_scalar_add(
                qkT_sb[:, 2, ts(0, 512)], psA[:, 1, :], bqk_sb[:, 2:3]
            )
            m, q2, pst = w1_streams[2]
            nc.vector.tensor_scalar_add(
                qkT_sb[:, m, ts(q2, 512)], pst[:], bqk_sb[:, m : m + 1]
            )
            nc.vector.tensor_scalar_add(
                qkT_sb[:, 0, ts(0, 512)], psA[:, 0, :], bqk_sb[:, 0:1]
            )
            for q2i, q2 in enumerate((0, 1)):
                pst = (psA, psB)[q2i]
                for mi, m in enumerate((0, 2)):
                    if q2 == 0:
                        continue  # psA evicted above
                    nc.vector.tensor_scalar_add(
                        qkT_sb[:, m, ts(q2, 512)],
                        pst[:, mi, :],
                        bqk_sb[:, m : m + 1],
                    )
            for si, (m, q2, pst) in enumerate(w1_streams):
                if si == 2:
                    continue  # evicted above
                nc.vector.tensor_scalar_add(
                    qkT_sb[:, m, ts(q2, 512)],
                    pst[:],
                    bqk_sb[:, m : m + 1],
                )

            # filler order: roughly when each unit is first needed; ensure()
            # guarantees correctness if the pace falls behind. hp=0 quarters
            # are all done in the ramp; v quarters go first (the jj=3 chunks
            # consume all of them), then hp=1 quarters in reverse-jj order.
            filler.add(("v", 0), gen_v_quarter(0))
            filler.add(("v", 1), gen_v_quarter(1))
            filler.add(("v", 2), gen_v_quarter(2))
            filler.add(("v", 3), gen_v_quarter(3))
            filler.add(("v", 4), gen_v_quarter(4))
            filler.add(("v", 5), gen_v_quarter(5))
            filler.add(("v", 6), gen_v_quarter(6))
            filler.add(("v", 7), gen_v_quarter(7))
            filler.add(("qk", 1, 3), gen_qk_quarter(1, 3))
            filler.add(("qk", 3, 0), gen_qk_quarter(3, 0))
            filler.add(("qk", 3, 1), gen_qk_quarter(3, 1))
            filler.add(("qk", 3, 2), gen_qk_quarter(3, 2))
            filler.add(("qk", 3, 3), gen_qk_quarter(3, 3))
            filler.add(("qk", 1, 2), gen_qk_quarter(1, 2))
            filler.add(("qk", 1, 1), gen_qk_quarter(1, 1))
            filler.add(("qk", 1, 0), gen_qk_quarter(1, 0))

            # Reverse-jj, hp-interleaved chunk order: the long ACT-heavy
            # jj=3 chunks run first, while the filler queue is full, so the
            # PE stays packed; the short jj=0 chunks run last, minimizing
            # the exposed endgame. proj units for q-rows [4jj,4jj+4) unlock
            # after each (1,jj) pair, spreading the output projection + DMA.
            for jj in (3, 2, 1, 0):
                attn_chunk(
                    0,
                    jj,
                    pull_scale=1.25,
                    start_pull=(0.0 if jj == 3 else 2600.0),
                )
                attn_chunk(
                    1, jj, pull_scale=1.25, start_pull=2600.0,
                    norm_split=(jj == 0),
                )
                for m in range(4 * jj, 4 * jj + 4):
                    filler.add(
                        ("proj", m), gen_proj(m, evict_on_act=(jj == 0))
                    )
            filler.drain()

    nc.finalize()
    _module_cache["nc"] = nc
    return nc


def _shard_inputs(x, w_qkv, b_qkv, w_proj):
    """Per-core input dicts. Core c: batch c//4, heads 4*(c%4) .. 4*(c%4)+3."""
    bf = ml_dtypes.bfloat16
    in_maps = []
    xTs = [np.ascontiguousarray(x[b].T).astype(bf) for b in range(B)]
    for c in range(N_CORES):
        b = c // GROUPS
        g = c % GROUPS
        qc = slice(256 * g, 256 * g + 256)
        kc = slice(D + 256 * g, D + 256 * g + 256)
        vc = slice(2 * D + 256 * g, 2 * D + 256 * g + 256)
        # 1/sqrt(hd) scale folded into the q columns of W and into b_q
        w_qk = np.ascontiguousarray(
            np.concatenate([w_qkv[:, qc] * SCALE, w_qkv[:, kc]], axis=1)
        ).astype(bf)
        bq = np.concatenate([b_qkv[qc] * SCALE, b_qkv[kc]]).astype(np.float32)
        b_qk = np.ascontiguousarray(bq.reshape(4, 128).T)
        w_v = np.ascontiguousarray(w_qkv[:, vc]).astype(bf)
        b_v = np.ascontiguousarray(np.broadcast_to(b_qkv[vc], (128, 256))).astype(
            np.float32
        )
        w_pr = np.ascontiguousarray(w_proj[256 * g : 256 * g + 256, :]).astype(bf)
        in_maps.append(
            {
                "xT": xTs[b],
                "w_qk": w_qk,
                "b_qk": b_qk,
                "w_v": w_v,
                "b_v": b_v,
                "w_pr": w_pr,
            }
        )
    return in_maps


def kernel(x, w_qkv, b_qkv, w_proj, b_proj, _spmd_kwargs=None):
    from concourse.bass_utils import run_bass_kernel_spmd

    x = np.asarray(x, dtype=np.float32)
    w_qkv = np.asarray(w_qkv, dtype=np.float32)
    b_qkv = np.asarray(b_qkv, dtype=np.float32)
    w_proj = np.asarray(w_proj, dtype=np.float32)
    b_proj = np.asarray(b_proj, dtype=np.float32)

    nc = _build_module()
    in_maps = _shard_inputs(x, w_qkv, b_qkv, w_proj)
    res = run_bass_kernel_spmd(
        nc, in_maps, list(range(N_CORES)), **(_spmd_kwargs or {})
    )
    out = np.empty((B, S, D), dtype=np.float32)
    for b in range(B):
        acc = np.zeros((S, D), dtype=np.float64)
        for g in range(GROUPS):
            acc += np.asarray(res.results[b * GROUPS + g]["y"], dtype=np.float64)
        out[b] = (acc + b_proj.astype(np.float64)).astype(np.float32)
    if _spmd_kwargs:
        kernel.last_result = res
    return out



# revision 51
# speedup vs baseline: 1.0114x; 1.0114x over previous
"""Causal self-attention (B=2, S=2048, D=1024, H=16, Hd=64) on 8 TRN2 NeuronCores.

Sharding: tensor-parallel over heads (4 heads/core) x data-parallel over batch
(cores 0-3 -> batch 0, cores 4-7 -> batch 1). Each core:
  - computes q^T,k^T (transposed layout, heads stacked in pairs on partitions)
    and v (natural layout) for its 4 heads
  - runs causal attention in transposed-score layout (scores_T[k, q]) so no
    transposes are ever needed; the v tiles carry 64 identical ones-columns,
    so each attnV matmul deposits 64 broadcast copies of the softmax
    denominator on PSUM partitions 0-63 for free -- normalization is then a
    single fast-approx DVE reciprocal over [64,512] + one DVE multiply
  - computes its partial output projection y_part = out_heads @ W_proj[rows]
Host sums the 4 bf16 partials per batch and adds b_proj (the unshard step for
a row-parallel matmul). Matmul datapath is bf16 (fp32 PSUM accumulation).

Schedule: software-pipelined around two hardware behaviors measured in the
NTFF traces: (1) the PE's HAM clock gate halves the clock after any ~idle
window, so the schedule keeps PE activity continuous -- dummy warm-up/
warm-keeper matmuls bridge the DMA load window and each chunk-boundary
normalize lull; (2) cross-engine round-trips (attnV PSUM WAR on the previous
chunk's normalize) are covered by "filler" units -- the qkv / v / output-
projection matmuls split into quarter-granular generators interleaved
between attention steps, paced by estimated PE-ns. attnV trails its exp by
three steps so it never waits on ScalarE. All eight hp=0 qk quarters are
computed during the input-DMA window (two accumulation waves, one stream per
PSUM bank, wave 1 trailing by one k-tile), so dense attention starts the
moment the load finishes, already at the warm 2.4 GHz clock. The long
(jj=3) chunks run first while filler is plentiful; the short jj=0 chunks
form the tail, with their projection evictions moved to the then-idle
ScalarE and y stored as full 2 KB rows.
"""

import sys

if "/opt/trn_rl_repo" not in sys.path:
    sys.path.insert(0, "/opt/trn_rl_repo")

import ml_dtypes
import numpy as np


def _ensure_axon_hooks():
    """bass_utils imports antenv.axon_hooks when tracing is requested; the
    slim agent image lacks it. Provide the real ctypes hook if possible,
    else a None-returning stub (bass_utils then skips tracing gracefully)."""
    try:
        import antenv.axon_hooks  # noqa: F401

        return
    except ImportError:
        pass
    import types

    hook = None
    try:
        from trn_agent_boot.trn_boot import _ntff_profile_via_ctypes

        hook = _ntff_profile_via_ctypes("/opt/axon/libaxon_pjrt.so")
    except Exception:
        pass
    mod = types.ModuleType("antenv.axon_hooks")
    mod.get_axon_ntff_profile_hook = lambda: hook
    mod.set_axon_ntff_profile_hook = lambda h: None
    sys.modules["antenv.axon_hooks"] = mod


_ensure_axon_hooks()

D = 1024
S = 2048
B = 2
H = 16
HD = 64
N_CORES = 8
GROUPS = 4  # cores per batch
HPC = 4  # heads per core
SCALE = 1.0 / np.sqrt(HD)
KT = D // 128  # 8 contraction tiles
ST = S // 128  # 16 seq tiles

_module_cache = {}


def _build_module():
    if "nc" in _module_cache:
        return _module_cache["nc"]

    import concourse.bacc as bacc
    import concourse.mybir as mybir
    import concourse.tile as tile
    from concourse.bass import ts

    f32 = mybir.dt.float32
    bf16 = mybir.dt.bfloat16
    AF = mybir.ActivationFunctionType

    nc = bacc.Bacc("TRN2", target_bir_lowering=False, debug=False)

    xT = nc.dram_tensor("xT", [D, S], bf16, kind="ExternalInput")
    w_qk = nc.dram_tensor("w_qk", [D, 512], bf16, kind="ExternalInput")
    b_qk = nc.dram_tensor("b_qk", [128, 4], f32, kind="ExternalInput")
    w_v = nc.dram_tensor("w_v", [D, 256], bf16, kind="ExternalInput")
    b_v = nc.dram_tensor("b_v", [128, 256], f32, kind="ExternalInput")
    w_pr = nc.dram_tensor("w_pr", [256, D], bf16, kind="ExternalInput")
    y = nc.dram_tensor("y", [S, D], bf16, kind="ExternalOutput")

    import contextlib

    with tile.TileContext(nc) as tc:
        with contextlib.ExitStack() as ctx2:
            const = ctx2.enter_context(tc.tile_pool(name="const", bufs=1))
            # ---- resident SBUF tensors ----
            xT_sb = const.tile([128, KT, S], bf16)
            wqk_sb = const.tile([128, KT, 512], bf16)
            wv_sb = const.tile([128, KT, 256], bf16)
            bqk_sb = const.tile([128, 4], f32)
            bv_sb = const.tile([128, 4, 64], f32)
            wpr_sb = const.tile([128, 2, D], bf16)
            ones_sb = const.tile([1, 64], f32)
            warm_sb = const.tile([1, 64], f32)
            qkT_sb = const.tile([128, 4, S], bf16)  # m: q01,q23,k01,k23
            wu_sb = const.tile([128, 512], bf16)  # HAM warmup operand
            v_sb = const.tile([128, ST, 4, 128], bf16)  # per head: [ones|63 pad|V]
            oT_sb = const.tile([128, 2, S], bf16)  # normalized attn out

            # The input load is issue-limited on a single queue (measured
            # ~290GB/s avg vs ~400GB/s fabric peak): split wqk/xT across the
            # sync + scalar queues (alternate k-tiles), wv behind on both,
            # wpr last. ACT queue is idle during the load so borrowing it is
            # free; exp work only starts ~28us in.
            nc.vector.memset(ones_sb[:], 1.0)
            nc.vector.memset(wu_sb[:], 0.03)
            # preload the ACT exp table set early, off the critical path
            nc.scalar.activation(warm_sb[:], ones_sb[:], AF.Exp)
            nc.sync.dma_start(out=bqk_sb[:], in_=b_qk[:])
            nc.sync.dma_start(out=bv_sb[:], in_=b_v[:])
            qs = (nc.sync, nc.scalar)
            for k in range(KT):
                qs[k % 2].dma_start(out=wqk_sb[:, k, :], in_=w_qk[ts(k, 128), :])
                qs[(k + 1) % 2].dma_start(out=xT_sb[:, k, :], in_=xT[ts(k, 128), :])
            for k in range(KT):
                qs[k % 2].dma_start(out=wv_sb[:, k, :], in_=w_v[ts(k, 128), :])
            nc.sync.dma_start(out=wpr_sb[:, 0, :], in_=w_pr[0:128, :])
            nc.scalar.dma_start(out=wpr_sb[:, 1, :], in_=w_pr[128:256, :])
            for h in range(HPC):
                # 64 identical ones-columns: the attnV matmul then deposits
                # 64 copies of the softmax denominator on PSUM partitions
                # 0-63 (same streamed column count, so zero extra PE cost),
                # and V lands on partitions 64-127. Normalization is then
                # just reciprocal_approx_fast over [64,512] + one DVE mul --
                # no partition_broadcast (gpsimd scheduling is erratic) and
                # no PSUM+PSUM operand pairs.
                nc.gpsimd.memset(v_sb[:, :, h, 0:64], 1.0)

            # PSUM budget (8 banks): scores 2 bufs x [128,2,512] = 4 banks,
            # attnV accumulators 2 x [65,512] = 2 banks, filler [128,1024]
            # = 2 banks.
            psS = ctx2.enter_context(tc.tile_pool(name="psS", bufs=2, space="PSUM"))
            psO = ctx2.enter_context(tc.tile_pool(name="psO", bufs=1, space="PSUM"))
            psF = ctx2.enter_context(tc.tile_pool(name="psF", bufs=2, space="PSUM"))
            ptp = ctx2.enter_context(tc.tile_pool(name="pt", bufs=6))
            ysbp = ctx2.enter_context(tc.tile_pool(name="ysb", bufs=3))
            nrm = ctx2.enter_context(tc.tile_pool(name="nrm", bufs=2))

            # ---- filler units: generators yielding approx PE-ns per matmul
            def gen_qk_quarter(m, q2):
                """qkT_sb[:, m, q2*512:...] = (x @ w_qk[:, m-tile]) + bias."""
                ps = psF.tile([128, 512], f32, tag="f", name="ps_qk")
                for k in range(KT):
                    nc.tensor.matmul(
                        ps[:, 0:512],
                        lhsT=wqk_sb[:, k, ts(m, 128)],
                        rhs=xT_sb[:, k, ts(q2, 512)],
                        start=(k == 0),
                        stop=(k == KT - 1),
                    )
                    yield 216.0
                nc.vector.tensor_scalar_add(
                    qkT_sb[:, m, ts(q2, 512)], ps[:, 0:512], bqk_sb[:, m : m + 1]
                )
                yield 0.0

            def gen_v_quarter(qt):
                """v_sb seq-tiles 2qt, 2qt+1 (natural layout, + bias)."""
                ps = psF.tile([128, 2, 4, 64], f32, tag="f", name="ps_v")
                for sti in range(2):
                    st = qt * 2 + sti
                    for k in range(KT):
                        nc.tensor.matmul(
                            ps[:, sti],
                            lhsT=xT_sb[:, k, ts(st, 128)],
                            rhs=wv_sb[:, k, :],
                            start=(k == 0),
                            stop=(k == KT - 1),
                        )
                        yield 110.0
                    nc.vector.tensor_add(
                        v_sb[:, st, :, 64:128], ps[:, sti], bv_sb[:]
                    )
                    yield 0.0

            def gen_proj(m, evict_on_act=False):
                """y[m-tile, :] = oT[m-tile]^T @ w_pr. Both 512-col halves
                land in one [128,1024] staging tile so the store is a single
                DMA with full 2KB DRAM lines (512-col stores ran at ~half
                fabric rate on 1KB lines)."""
                y_sb = ysbp.tile([128, 1024], bf16, tag="ysb", name="y_sb")
                for nch in range(2):
                    ps = psF.tile([128, 512], f32, tag="f", name="ps_y")
                    for kp in range(2):
                        nc.tensor.matmul(
                            ps[:, 0:512],
                            lhsT=oT_sb[:, kp, ts(m, 128)],
                            rhs=wpr_sb[:, kp, ts(nch, 512)],
                            start=(kp == 0),
                            stop=(kp == 1),
                        )
                        yield 216.0
                    if evict_on_act:
                        # late proj units run while ACT is idle and DVE is
                        # the endgame bottleneck (normalize muls + evictions)
                        nc.scalar.activation(y_sb[:, ts(nch, 512)], ps[:], AF.Copy)
                    else:
                        nc.vector.tensor_scalar_add(
                            y_sb[:, ts(nch, 512)], ps[:], 0.0
                        )
                nc.sync.dma_start(out=y[ts(m, 128), :], in_=y_sb[:])
                yield 0.0

            class Filler:
                """Queue of named filler units (generators of PE matmuls).
                pull(ns) paces emission by approximate PE time; ensure(key)
                force-emits a unit NOW (producers must be emitted before
                their consumers — units are mutually independent)."""

                def __init__(self):
                    self.queue = []  # list of (key, gen)
                    self.cur = None  # (key, gen) partially emitted

                def add(self, key, gen):
                    self.queue.append((key, gen))

                def ensure(self, key):
                    if self.cur is not None and self.cur[0] == key:
                        for _ in self.cur[1]:
                            pass
                        self.cur = None
                        return
                    for i, (k, g) in enumerate(self.queue):
                        if k == key:
                            del self.queue[i]
                            for _ in g:
                                pass
                            return

                def pull(self, ns):
                    while ns > 0:
                        if self.cur is None:
                            if not self.queue:
                                return
                            self.cur = self.queue.pop(0)
                        try:
                            ns -= next(self.cur[1])
                        except StopIteration:
                            self.cur = None

                def drain(self):
                    while self.cur is not None or self.queue:
                        self.pull(1e12)

            filler = Filler()

            # ---- attention chunk: 512 q-cols of one head pair ----
            def attn_chunk(
                hp, jj, pull_scale, start_pull=0.0, norm_split=False,
                add_after_start=(),
            ):
                c0 = 512 * jj
                n_sk = 4 * jj + 4
                po = [
                    psO.tile([128, 512], f32, tag=f"o{h}", name=f"po{h}")
                    for h in range(2)
                ]

                def emit_attnv(sk, pt, col0, n):
                    for h in range(2):
                        nc.tensor.matmul(
                            po[h][:, col0 - c0 : col0 - c0 + n],
                            lhsT=v_sb[:, sk, 2 * hp + h, :],
                            rhs=pt[:, h, 0:n],
                            start=(sk == 0),
                            stop=(sk == n_sk - 1),
                        )

                filler.ensure(("qk", hp, jj))  # this chunk's q quarter
                filler.ensure(("qk", 2 + hp, 0))  # first k quarter
                # Boundary burst: the first attnV of this chunk reuses the
                # previous chunk's po PSUM banks (psO bufs=1), so it WAR-waits
                # on the prev normalize chain (reciprocal -> mul on DVE).
                # Emit enough filler ahead of it that the PE never drains
                # while that chain runs. The previous pair's proj units are
                # only added AFTER this burst: their oT LDWEIGHTS waits on
                # the muls just emitted, so pulling one inside the burst
                # would stall the PE queue on the very chain we're covering.
                filler.pull(start_pull)
                for key, gen in add_after_start:
                    filler.add(key, gen)
                pend = []
                for sk in range(n_sk):
                    col0 = max(c0, sk * 128)
                    n = c0 + 512 - col0
                    filler.ensure(("qk", 2 + hp, sk // 4))  # k quarter
                    ps = psS.tile([128, 2, 512], f32, tag="s", name="ps")
                    for h in range(2):
                        hr = h * 64
                        nc.tensor.matmul(
                            ps[:, h, 0:n],
                            lhsT=qkT_sb[hr : hr + 64, 2 + hp, ts(sk, 128)],
                            rhs=qkT_sb[hr : hr + 64, hp, col0 : col0 + n],
                            start=True,
                            stop=True,
                            tile_position=(hr, 0),
                        )
                    pt = ptp.tile([128, 2, 512], bf16, tag="pt", name="pt")
                    if n == 512:
                        nc.scalar.activation(pt[:], ps[:], AF.Exp)
                    else:
                        nc.scalar.activation(pt[:, :, 0:n], ps[:, :, 0:n], AF.Exp)
                    if col0 == sk * 128:
                        # causal mask: zero pt below the diagonal on the
                        # (idle) gpsimd engine; the one-step delay before
                        # attnV consumes pt hides the latency, and it keeps
                        # the 64 mask matmuls off the bottleneck PE.
                        for h in range(2):
                            nc.gpsimd.affine_select(
                                out=pt[:, h, 0:128],
                                in_=pt[:, h, 0:128],
                                compare_op=mybir.AluOpType.is_ge,
                                fill=0.0,
                                base=0,
                                pattern=[[1, 128]],
                                channel_multiplier=-1,
                            )
                    # filler between this step's scores and the previous
                    # step's attnV: covers the exp latency on the PE queue.
                    act_ns = (2 * n + 352) / 1.2
                    attn_pe = (3 * n) / 2.4 + 60
                    filler.pull(max(150.0, (act_ns - attn_pe) * pull_scale))
                    # two-step attnV delay: attnV for step N is emitted
                    # during step N+2, so it never waits on the exp stream,
                    # and a chunk's first attnV sits two steps past the
                    # boundary (more slack for the po WAR on the previous
                    # chunk's normalize).
                    if len(pend) == 3:
                        p = pend.pop(0)
                        filler.ensure(("v", p[0] // 2))
                        emit_attnv(*p)
                    pend.append((sk, pt, col0, n))
                for p in pend:
                    filler.ensure(("v", p[0] // 2))
                    emit_attnv(*p)
                # A filler slab BEFORE the normalize ops: these fillers'
                # DVE evictions enter the queue ahead of rec/mul (which wait
                # on the attnV just emitted), so the psF-bank WAR for the
                # next fillers never stalls the PE behind a blocked DVE head.
                filler.pull(1000.0)
                # normalization: partitions 0-63 of po hold 64 identical
                # copies of the denominator (ones-columns in the v tile), so
                # 1/denom lands broadcast-ready in one reciprocal, and one
                # DVE mul (PSUM x SBUF) finishes the chunk. Deterministic
                # ~1.4us/head, all on the in-order DVE queue. For the final
                # pair (norm_split) the muls go out in 128-col slices,
                # head-interleaved, so each proj m-tile can start as soon as
                # its slice of oT is normalized.
                rbcs = []
                if norm_split:
                    for h in range(2):
                        rbc = nrm.tile([64, 512], f32, tag=f"rbc{h}", name="rbc")
                        nc.vector.reciprocal_approx_fast(
                            out=rbc[:], in_=po[h][0:64, :]
                        )
                        rbcs.append(rbc)
                    for wi in range(8):
                        nc.tensor.matmul(
                            po[wi % 2][0:64, :],
                            lhsT=wu_sb[:, 0:64],
                            rhs=wu_sb[:],
                            start=True,
                            stop=True,
                        )
                    for p4 in range(4):
                        sl = slice(128 * p4, 128 * (p4 + 1))
                        for h in range(2):
                            hr = h * 64
                            nc.vector.tensor_mul(
                                oT_sb[hr : hr + 64, hp, c0 + 128 * p4 : c0 + 128 * (p4 + 1)],
                                po[h][64:128, sl],
                                rbcs[h][:, sl],
                            )
                else:
                    for h in range(2):
                        rbc = nrm.tile([64, 512], f32, tag=f"rbc{h}", name="rbc")
                        nc.vector.reciprocal_approx_fast(
                            out=rbc[:], in_=po[h][0:64, :]
                        )
                        rbcs.append(rbc)
                    # HAM warm-keepers: dummy MMs into the dead denominator
                    # region (rec already consumed it). They WAR-wait the
                    # recs, so they execute exactly inside the boundary lull.
                    # Full 512-col streams: 64-col keepers pipeline at ~32ns
                    # apart and give only ~0.3us of activity, too little for
                    # the ~1.7us lull the HAM MID window sees.
                    for wi in range(6):
                        nc.tensor.matmul(
                            po[wi % 2][0:64, :],
                            lhsT=wu_sb[:, 0:64],
                            rhs=wu_sb[:],
                            start=True,
                            stop=True,
                        )
                    for h in range(2):
                        hr = h * 64
                        nc.vector.tensor_mul(
                            oT_sb[hr : hr + 64, hp, c0 : c0 + 512],
                            po[h][64:128, :],
                            rbcs[h][:],
                        )

            # ---- HAM warmup: dense dummy matmuls at t=0 (PE is otherwise
            # idle waiting on DMA, and would run cold at 1.2 GHz otherwise).
            ps_w = psF.tile([128, 512], f32, tag="f", name="ps_warm")
            for i in range(10):
                nc.tensor.matmul(
                    ps_w[:, 0:512],
                    lhsT=wu_sb[:, 0:128],
                    rhs=wu_sb[:],
                    start=(i == 0),
                    stop=(i == 9),
                )

            # ---- ramp: ALL 8 hp=0 qk quarters (q01 + k01 over the full
            # sequence), k-pipelined with the input DMA stream. Both waves
            # interleave per k-tile (8 MMs / ~1.7us of PE per ~1.6us k-tile
            # arrival) so the PE runs at ~full duty through the load window
            # and HAM never re-throttles. Wave 0 (q2 0,1) accumulates in the
            # psS banks; wave 1 (q2 2,3) in 256-col half-groups placed in
            # the otherwise-idle psO/psF banks (1 bank each). The psF buf0
            # group is emitted last within each k so its WAR on the warmup
            # accumulator clears before it reaches the queue head.
            # One accumulation stream per PSUM bank: interleaved start=True
            # groups sharing a bank corrupt each other (the has_written
            # clear is bank-wide), so each (m, q2) stream gets a whole bank.
            psA = psS.tile([128, 2, 512], f32, tag="s", name="rampA")
            psB = psS.tile([128, 2, 512], f32, tag="s", name="rampB")
            w1_streams = [
                (0, 2, psO.tile([128, 512], f32, tag="o0", name="rampO0")),
                (2, 2, psO.tile([128, 512], f32, tag="o1", name="rampO1")),
                (0, 3, psF.tile([128, 512], f32, tag="f", name="rampF1")),
                (2, 3, psF.tile([128, 512], f32, tag="f", name="rampF0")),
            ]
            # wave1 trails wave0 by 3 k-tiles: during the cold-clock window
            # only wave0 runs (4 cold MMs/k-tile ~= the DMA arrival rate);
            # once HAM is warm, 8 MMs/k-tile still matches arrivals, and the
            # last 3 wave1 k-groups run dense right after the load.
            def w0_mms(k):
                for q2i, q2 in enumerate((0, 1)):
                    pst = (psA, psB)[q2i]
                    for mi, m in enumerate((0, 2)):
                        nc.tensor.matmul(
                            pst[:, mi, :],
                            lhsT=wqk_sb[:, k, ts(m, 128)],
                            rhs=xT_sb[:, k, ts(q2, 512)],
                            start=(k == 0),
                            stop=(k == KT - 1),
                        )

            def w1_mms(k):
                for m, q2, pst in w1_streams:
                    nc.tensor.matmul(
                        pst[:],
                        lhsT=wqk_sb[:, k, ts(m, 128)],
                        rhs=xT_sb[:, k, ts(q2, 512)],
                        start=(k == 0),
                        stop=(k == KT - 1),
                    )

            for k in range(KT):
                w0_mms(k)
                if k >= 1:
                    w1_mms(k - 1)
            w1_mms(KT - 1)
            # evictions in first-needed order: chunk (0,3) sk=0 reads the
            # k01/q2=0 quarter (psA mi=1) as lhsT and the q01/q2=3 quarter
            # (wave1 stream 2) as rhs.
            nc.vector.tensor_scalar_add(
                qkT_sb[:, 2, ts(0, 512)], psA[:, 1, :], bqk_sb[:, 2:3]
            )
            m, q2, pst = w1_streams[2]
            nc.vector.tensor_scalar_add(
                qkT_sb[:, m, ts(q2, 512)], pst[:], bqk_sb[:, m : m + 1]
            )
            nc.vector.tensor_scalar_add(
                qkT_sb[:, 0, ts(0, 512)], psA[:, 0, :], bqk_sb[:, 0:1]
            )
            for q2i, q2 in enumerate((0, 1)):
                pst = (psA, psB)[q2i]
                for mi, m in enumerate((0, 2)):
                    if q2 == 0:
                        continue  # psA evicted above
                    nc.vector.tensor_scalar_add(
                        qkT_sb[:, m, ts(q2, 512)],
                        pst[:, mi, :],
                        bqk_sb[:, m : m + 1],
                    )
            for si, (m, q2, pst) in enumerate(w1_streams):
                if si == 2:
                    continue  # evicted above
                nc.vector.tensor_scalar_add(
                    qkT_sb[:, m, ts(q2, 512)],
                    pst[:],
                    bqk_sb[:, m : m + 1],
                )

            # filler order: roughly when each unit is first needed; ensure()
            # guarantees correctness if the pace falls behind. hp=0 quarters
            # are all done in the ramp; v quarters go first (the jj=3 chunks
            # consume all of them), then hp=1 quarters in reverse-jj order.
            filler.add(("v", 0), gen_v_quarter(0))
            filler.add(("v", 1), gen_v_quarter(1))
            filler.add(("v", 2), gen_v_quarter(2))
            filler.add(("v", 3), gen_v_quarter(3))
            filler.add(("v", 4), gen_v_quarter(4))
            filler.add(("v", 5), gen_v_quarter(5))
            filler.add(("v", 6), gen_v_quarter(6))
            filler.add(("v", 7), gen_v_quarter(7))
            filler.add(("qk", 1, 3), gen_qk_quarter(1, 3))
            filler.add(("qk", 3, 0), gen_qk_quarter(3, 0))
            filler.add(("qk", 3, 1), gen_qk_quarter(3, 1))
            filler.add(("qk", 3, 2), gen_qk_quarter(3, 2))
            filler.add(("qk", 3, 3), gen_qk_quarter(3, 3))
            filler.add(("qk", 1, 2), gen_qk_quarter(1, 2))
            filler.add(("qk", 1, 1), gen_qk_quarter(1, 1))
            filler.add(("qk", 1, 0), gen_qk_quarter(1, 0))

            # Reverse-jj, hp-interleaved chunk order: the long ACT-heavy
            # jj=3 chunks run first, while the filler queue is full, so the
            # PE stays packed; the short jj=0 chunks run last, minimizing
            # the exposed endgame. proj units for q-rows [4jj,4jj+4) unlock
            # after each (1,jj) pair, spreading the output projection + DMA.
            for jj in (3, 2, 1, 0):
                attn_chunk(
                    0,
                    jj,
                    pull_scale=1.25,
                    start_pull=(0.0 if jj == 3 else 2600.0),
                )
                attn_chunk(
                    1, jj, pull_scale=1.25, start_pull=2600.0,
                    norm_split=(jj == 0),
                )
                for m in range(4 * jj, 4 * jj + 4):
                    filler.add(
                        ("proj", m), gen_proj(m, evict_on_act=(jj == 0))
                    )
            filler.drain()

    nc.finalize()
    _module_cache["nc"] = nc
    return nc


def _shard_inputs(x, w_qkv, b_qkv, w_proj):
    """Per-core input dicts. Core c: batch c//4, heads 4*(c%4) .. 4*(c%4)+3."""
    bf = ml_dtypes.bfloat16
    in_maps = []
    xTs = [np.ascontiguousarray(x[b].T).astype(bf) for b in range(B)]
    for c in range(N_CORES):
        b = c // GROUPS
        g = c % GROUPS
        qc = slice(256 * g, 256 * g + 256)
        kc = slice(D + 256 * g, D + 256 * g + 256)
        vc = slice(2 * D + 256 * g, 2 * D + 256 * g + 256)
        # 1/sqrt(hd) scale folded into the q columns of W and into b_q
        w_qk = np.ascontiguousarray(
            np.concatenate([w_qkv[:, qc] * SCALE, w_qkv[:, kc]], axis=1)
        ).astype(bf)
        bq = np.concatenate([b_qkv[qc] * SCALE, b_qkv[kc]]).astype(np.float32)
        b_qk = np.ascontiguousarray(bq.reshape(4, 128).T)
        w_v = np.ascontiguousarray(w_qkv[:, vc]).astype(bf)
        b_v = np.ascontiguousarray(np.broadcast_to(b_qkv[vc], (128, 256))).astype(
            np.float32
        )
        w_pr = np.ascontiguousarray(w_proj[256 * g : 256 * g + 256, :]).astype(bf)
        in_maps.append(
            {
                "xT": xTs[b],
                "w_qk": w_qk,
                "b_qk": b_qk,
                "w_v": w_v,
                "b_v": b_v,
                "w_pr": w_pr,
            }
        )
    return in_maps


def kernel(x, w_qkv, b_qkv, w_proj, b_proj, _spmd_kwargs=None):
    from concourse.bass_utils import run_bass_kernel_spmd

    x = np.asarray(x, dtype=np.float32)
    w_qkv = np.asarray(w_qkv, dtype=np.float32)
    b_qkv = np.asarray(b_qkv, dtype=np.float32)
    w_proj = np.asarray(w_proj, dtype=np.float32)
    b_proj = np.asarray(b_proj, dtype=np.float32)

    nc = _build_module()
    in_maps = _shard_inputs(x, w_qkv, b_qkv, w_proj)
    res = run_bass_kernel_spmd(
        nc, in_maps, list(range(N_CORES)), **(_spmd_kwargs or {})
    )
    out = np.empty((B, S, D), dtype=np.float32)
    for b in range(B):
        acc = np.zeros((S, D), dtype=np.float64)
        for g in range(GROUPS):
            acc += np.asarray(res.results[b * GROUPS + g]["y"], dtype=np.float64)
        out[b] = (acc + b_proj.astype(np.float64)).astype(np.float32)
    if _spmd_kwargs:
        kernel.last_result = res
    return out

